# revision 2
# baseline (speedup 1.0000x reference)
"""Trainium2 Bass kernel for nn_CorrelationBlock.

Reference computation (B=32, H=64, W=64, D=64, X=Y=Z=32):
    fsum = fk.sum(-1)                                  # (B, H, W)
    corr = einsum('bxyz,bhw->hwxyz', ok, fsum)         # (H, W, X, Y, Z)
    corr = corr.reshape(B, H, W, 1024)                 # flat reinterpret
    ak   = corr.sum(axis=2)                            # (B, H, 1024)
    avg  = ak.mean()
    akh  = where(ak > avg, ak, 0)
    akh  = akh - akh.min(axis=1, keepdims=True)
    akh  = akh / akh.max(axis=1, keepdims=True)
    returns (corr, akh)

Sharding: H is split into 8 slabs of 8; core i computes corr rows
h in [8i, 8i+8) -- exactly corr.reshape(...)[4i:4i+4] -- plus the matching
ak rows.  ak factorizes without touching the 512MB corr tensor:
    ak[b2,h2,yz] = sum_b (fsum[b,h,2q] + fsum[b,h,2q+1]) * (sum_x ok[b,x,yz])
with h = 2*b2 + h2//32, q = h2%32.  The global mean over ak reduces to
sum_b (sum_hw fsum) * (sum_xyz ok), accumulated with one scalar AllReduce.
"""

import numpy as np

import concourse.bass as bass
import concourse.mybir as mybir
import concourse.tile as tile
from concourse import bacc
from concourse.bass_utils import run_bass_kernel_spmd

F32 = mybir.dt.float32
NCORES = 8
B, H, W, D = 32, 64, 64, 64
HSLAB = H // NCORES          # 8 h-values per core
KVOL = 32 * 32               # y*z = 1024
XYZ = 32 * KVOL              # 32768
AK_COUNT = float(B * H * KVOL)  # 2_097_152 elements in ak

LAST_RESULTS = None   # test harness introspection


def _build_kernel():
    nc = bacc.Bacc("TRN2", target_bir_lowering=False, num_devices=NCORES)

    # ---- I/O ------------------------------------------------------------
    # okB: ok rearranged host-side to partitions p = xh*32 + b (xh = x//8),
    #      free = (x%8)*1024 + y*32 + z.  Serves as matmul rhs (4 partition
    #      bands of 32 = 4 column chunks of 8192) and as the source of oksum.
    okB = nc.dram_tensor("okB", [128, 8192], F32, kind="ExternalInput")
    # fkT: this core's fk slab, partitions p = b*4 + (h_loc//2),
    #      free = (h_loc%2)*4096 + w*64 + d.
    fkT = nc.dram_tensor("fkT", [128, 8192], F32, kind="ExternalInput")
    # selector constants (see host side)
    sel4r = nc.dram_tensor("sel4r", [4, 128, 128], F32, kind="ExternalInput")
    selb = nc.dram_tensor("selb", [128, 32], F32, kind="ExternalInput")
    ident = nc.dram_tensor("ident", [128, 128], F32, kind="ExternalInput")

    # corr slab, rows = h_loc*64 + w, cols = xyz
    corr_out = nc.dram_tensor("corr_out", [512, 32768], F32, kind="ExternalOutput")
    # ak_hat slab, rows = h_loc*32 + q  (== b2_loc*64 + h2), cols = y*32+z
    akhat_out = nc.dram_tensor("akhat_out", [256, 1024], F32, kind="ExternalOutput")

    with tile.TileContext(nc) as tc:
        with (
            tc.tile_pool(name="bigio", bufs=1) as bigio,
            tc.tile_pool(name="consts", bufs=1) as consts,
            tc.tile_pool(name="small", bufs=1) as small,
            tc.tile_pool(name="stage", bufs=3) as stage_pool,
            tc.tile_pool(name="cps", bufs=4, space="PSUM") as cps,
            tc.tile_pool(name="sps", bufs=2, space="PSUM") as sps,
            tc.tile_pool(name="dram", bufs=1, space="DRAM") as dram_pool,
        ):
            # ---- Phase A: loads ----------------------------------------
            okB_sb = bigio.tile([128, 8192], F32, name="okB_sb")
            nc.sync.dma_start(okB_sb[:], okB[:])
            fkT_sb = bigio.tile([128, 8192], F32, name="fkT_sb")
            nc.sync.dma_start(fkT_sb[:], fkT[:])

            sel_sb = consts.tile([128, 4, 128], F32, name="sel_sb")
            nc.sync.dma_start(sel_sb[:], sel4r.rearrange("h p m -> p h m"))
            selb_sb = consts.tile([128, 32], F32, name="selb_sb")
            nc.sync.dma_start(selb_sb[:], selb[:])
            ident_sb = consts.tile([128, 128], F32, name="ident_sb")
            nc.sync.dma_start(ident_sb[:], ident[:])

            # ---- Phase B: fsum / oksum ---------------------------------
            # fsumT[p=(b,hh), g=(hl,w)] = sum_d fkT[p, g*64 + d]
            fsumT = small.tile([128, 128], F32, name="fsumT")
            nc.vector.tensor_reduce(
                fsumT[:],
                fkT_sb.rearrange("p (g d) -> p g d", d=64),
                axis=mybir.AxisListType.X,
                op=mybir.AluOpType.add,
            )
            # okS1[p=(xh,b), k] = sum_xl okB[p, xl*1024 + k]
            okS1 = small.tile([128, 1024], F32, name="okS1")
            nc.vector.tensor_reduce(
                okS1[:],
                okB_sb.rearrange("p (xl k) -> p k xl", xl=8),
                axis=mybir.AxisListType.X,
                op=mybir.AluOpType.add,
            )

            # fsum_rep[(xh', b), hloc*64 + w] = fsum[b, hloc, w] for every xh'
            fsum_rep = small.tile([128, 512], F32, name="fsum_rep")
            for hh in range(4):
                fsr_ps = sps.tile([128, 512], F32, name="sps_t", tag="sps")
                nc.tensor.matmul(
                    fsr_ps[:, 0:128],
                    sel_sb[:, hh, :],
                    fsumT[:],
                    start=True,
                    stop=True,
                )
                nc.any.tensor_copy(
                    out=fsum_rep[:, hh * 128 : (hh + 1) * 128], in_=fsr_ps[:, 0:128]
                )

            # oksum[b, k] = sum_x ok[b, x, yz]   (32 partitions)
            oksum_sb = small.tile([32, 1024], F32, name="oksum_sb")
            for j in range(2):
                okm_ps = sps.tile([128, 512], F32, name="sps_t", tag="sps")
                nc.tensor.matmul(
                    okm_ps[0:32, :],
                    selb_sb[:],
                    okS1[:, j * 512 : (j + 1) * 512],
                    start=True,
                    stop=True,
                )
                nc.any.tensor_copy(
                    out=oksum_sb[:, j * 512 : (j + 1) * 512], in_=okm_ps[0:32, :]
                )

            # ---- Phase C: global mean via one scalar AllReduce ----------
            # fpair[b, hloc*32+q] = fsum[b,hloc,2q] + fsum[b,hloc,2q+1]
            fpair_sb = small.tile([32, 256], F32, name="fpair_sb")
            fsum_pairs = fsum_rep[0:32].rearrange("p (hq two) -> p hq two", two=2)
            nc.vector.tensor_add(
                fpair_sb[:], fsum_pairs[:, :, 0], fsum_pairs[:, :, 1]
            )

            okt = small.tile([32, 1], F32, name="okt")
            nc.vector.tensor_reduce(
                okt[:], oksum_sb[:], axis=mybir.AxisListType.X, op=mybir.AluOpType.add
            )
            fst = small.tile([32, 1], F32, name="fst")
            nc.vector.tensor_reduce(
                fst[:], fsum_rep[0:32], axis=mybir.AxisListType.X,
                op=mybir.AluOpType.add,
            )
            prod = small.tile([32, 1], F32, name="prod")
            nc.vector.tensor_mul(prod[:], okt[:], fst[:])

            ones32 = small.tile([32, 1], F32, name="ones32")
            nc.vector.memset(ones32[:], 1.0)
            pi_ps = sps.tile([128, 512], F32, name="sps_t", tag="sps")
            nc.tensor.matmul(pi_ps[0:1, 0:1], prod[:], ones32[:], start=True, stop=True)
            pi_sb = small.tile([1, 1], F32, name="pi_sb")
            nc.any.tensor_copy(out=pi_sb[:], in_=pi_ps[0:1, 0:1])

            cc_in = dram_pool.tile([1, 1], F32, name="cc_in")
            cc_out = dram_pool.tile([1, 1], F32, name="cc_out")
            nc.sync.dma_start(cc_in[:], pi_sb[:])
            nc.gpsimd.collective_compute(
                "AllReduce",
                mybir.AluOpType.add,
                replica_groups=[list(range(NCORES))],
                ins=[cc_in.opt()],
                outs=[cc_out.opt()],
            )
            tot_sb = small.tile([1, 1], F32, name="tot_sb")
            nc.sync.dma_start(tot_sb[:], cc_out[:])

            avg1 = small.tile([1, 1], F32, name="avg1")
            nc.vector.tensor_scalar_mul(avg1[:], tot_sb[:], 1.0 / AK_COUNT)
            ones_r = small.tile([1, 128], F32, name="ones_r")
            nc.vector.memset(ones_r[:], 1.0)
            avgb_ps = sps.tile([128, 512], F32, name="sps_t", tag="sps")
            nc.tensor.matmul(
                avgb_ps[:, 0:1], ones_r[:], avg1[:], start=True, stop=True
            )
            avg_bc = small.tile([128, 1], F32, name="avg_bc")
            nc.any.tensor_copy(out=avg_bc[:], in_=avgb_ps[:, 0:1])

            # ---- Phase D: corr slab (the big streamed matmul) -----------
            for xh in range(4):
                pbase = 32 * xh
                for m in range(4):
                    lhsT = fsum_rep[pbase : pbase + 32, m * 128 : (m + 1) * 128]
                    for s in range(2):
                        stg = stage_pool.tile(
                            [128, 4096], F32, name="stg", tag="stg"
                        )
                        for j in range(8):
                            ns = s * 8 + j
                            cp = cps.tile([128, 512], F32, name="cp", tag="cp")
                            nc.tensor.matmul(
                                cp[:],
                                lhsT,
                                okB_sb[pbase : pbase + 32, ns * 512 : (ns + 1) * 512],
                                start=True,
                                stop=True,
                                tile_position=(pbase, 0),
                            )
                            if j % 2 == 0:
                                nc.scalar.copy(
                                    out=stg[:, j * 512 : (j + 1) * 512], in_=cp[:]
                                )
                            else:
                                nc.vector.tensor_copy(
                                    out=stg[:, j * 512 : (j + 1) * 512], in_=cp[:]
                                )
                        nc.sync.dma_start(
                            corr_out[
                                m * 128 : (m + 1) * 128,
                                xh * 8192 + s * 4096 : xh * 8192 + (s + 1) * 4096,
                            ],
                            stg[:],
                        )

            # ---- Phase E: ak_hat ---------------------------------------
            akh_r0 = small.tile([128, 1024], F32, name="akh_r0")
            akh_r1 = small.tile([128, 1024], F32, name="akh_r1")
            for kc in range(8):
                akT_ps = sps.tile([128, 512], F32, name="sps_t", tag="sps")
                nc.tensor.matmul(
                    akT_ps[:, 0:256],
                    oksum_sb[:, kc * 128 : (kc + 1) * 128],
                    fpair_sb[:],
                    start=True,
                    stop=True,
                )
                akT = small.tile([128, 256], F32, name="akT", tag="akT", bufs=2)
                nc.vector.tensor_copy(out=akT[:], in_=akT_ps[:, 0:256])

                mask = small.tile([128, 256], F32, name="mask", tag="mask", bufs=2)
                nc.vector.tensor_scalar(
                    mask[:], akT[:], avg_bc[:], None, op0=mybir.AluOpType.is_gt
                )
                nc.vector.tensor_mul(akT[:], akT[:], mask[:])

                grp = akT.rearrange("p (g h) -> p g h", h=64)
                mins = small.tile([128, 4], F32, name="mins", tag="mins", bufs=2)
                nc.vector.tensor_reduce(
                    mins[:], grp, axis=mybir.AxisListType.X, op=mybir.AluOpType.min
                )
                for g in range(4):
                    nc.vector.tensor_scalar(
                        akT[:, g * 64 : (g + 1) * 64],
                        akT[:, g * 64 : (g + 1) * 64],
                        mins[:, g : g + 1],
                        None,
                        op0=mybir.AluOpType.subtract,
                    )
                maxs = small.tile([128, 4], F32, name="maxs", tag="maxs", bufs=2)
                nc.vector.tensor_reduce(
                    maxs[:], grp, axis=mybir.AxisListType.X, op=mybir.AluOpType.max
                )
                rec = small.tile([128, 4], F32, name="rec", tag="rec", bufs=2)
                nc.vector.reciprocal(rec[:], maxs[:])
                for g in range(4):
                    nc.vector.tensor_scalar(
                        akT[:, g * 64 : (g + 1) * 64],
                        akT[:, g * 64 : (g + 1) * 64],
                        rec[:, g : g + 1],
                        None,
                        op0=mybir.AluOpType.mult,
                    )

                # transpose (k x row) -> (row x k) in two 128x128 blocks
                for half, dst in ((0, akh_r0), (1, akh_r1)):
                    t_ps = sps.tile([128, 512], F32, name="sps_t", tag="sps")
                    nc.tensor.transpose(
                        t_ps[:, 0:128],
                        akT[:, half * 128 : (half + 1) * 128],
                        ident_sb[:],
                    )
                    nc.any.tensor_copy(
                        out=dst[:, kc * 128 : (kc + 1) * 128], in_=t_ps[:, 0:128]
                    )

            nc.sync.dma_start(akhat_out[0:128, :], akh_r0[:])
            nc.sync.dma_start(akhat_out[128:256, :], akh_r1[:])

    nc.compile()
    return nc


_CACHE = {}


def _get_nc():
    if "nc" not in _CACHE:
        _CACHE["nc"] = _build_kernel()
    return _CACHE["nc"]


def _make_consts():
    sel4r = np.zeros((4, 128, 128), dtype=np.float32)
    b = np.arange(32)
    xh = np.arange(4)
    for hh in range(4):
        for x in xh:
            sel4r[hh, b * 4 + hh, x * 32 + b] = 1.0
    selb = np.tile(np.eye(32, dtype=np.float32), (4, 1))
    ident = np.eye(128, dtype=np.float32)
    return sel4r, selb, ident


def kernel(ok, fk):
    global LAST_RESULTS
    import os

    ok = np.ascontiguousarray(np.asarray(ok), dtype=np.float32)
    fk = np.ascontiguousarray(np.asarray(fk), dtype=np.float32)
    assert ok.shape == (B, 32, 32, 32) and fk.shape == (B, H, W, D)

    okB = np.ascontiguousarray(
        ok.reshape(32, 4, 8192).transpose(1, 0, 2).reshape(128, 8192)
    )
    sel4r, selb, ident = _make_consts()

    in_maps = []
    for i in range(NCORES):
        fkT = np.ascontiguousarray(fk[:, 8 * i : 8 * (i + 1)]).reshape(128, 8192)
        in_maps.append(
            {"okB": okB, "fkT": fkT, "sel4r": sel4r, "selb": selb, "ident": ident}
        )

    nc = _get_nc()
    res = run_bass_kernel_spmd(
        nc,
        in_maps,
        list(range(NCORES)),
        trace=bool(int(os.environ.get("KERNEL_TRACE", "0"))),
    )
    LAST_RESULTS = res

    corr = np.empty((64, 64 * 32768), dtype=np.float32)
    akh = np.empty((32, 64, 1024), dtype=np.float32)
    for i in range(NCORES):
        r = res.results[i]
        corr[8 * i : 8 * (i + 1)] = r["corr_out"].reshape(8, 64 * 32768)
        akh[4 * i : 4 * (i + 1)] = r["akhat_out"].reshape(4, 64, 1024)
    corr = corr.reshape(B, H, W, 1024)
    return corr, akh


# revision 3
# speedup vs baseline: 1.8468x; 1.8468x over previous
"""Trainium2 Bass kernel for nn_CorrelationBlock.

Reference computation (B=32, H=64, W=64, D=64, X=Y=Z=32):
    fsum = fk.sum(-1)                                  # (B, H, W)
    corr = einsum('bxyz,bhw->hwxyz', ok, fsum)         # (H, W, X, Y, Z)
    corr = corr.reshape(B, H, W, 1024)                 # flat reinterpret
    ak   = corr.sum(axis=2)                            # (B, H, 1024)
    avg  = ak.mean()
    akh  = where(ak > avg, ak, 0)
    akh  = akh - akh.min(axis=1, keepdims=True)
    akh  = akh / akh.max(axis=1, keepdims=True)
    returns (corr, akh)

Sharding: H is split into 8 slabs of 8; core i computes corr rows
h in [8i, 8i+8) -- exactly corr.reshape(...)[4i:4i+4] -- plus the matching
ak rows.  ak factorizes without touching the 512MB corr tensor:
    ak[b2,h2,yz] = sum_b (fsum[b,h,2q] + fsum[b,h,2q+1]) * (sum_x ok[b,x,yz])
with h = 2*b2 + h2//32, q = h2%32.  The global mean over ak reduces to
sum_b (sum_hw fsum) * (sum_xyz ok), accumulated with one scalar AllReduce.

The corr matmul has contraction K=32 (the batch dim), so the four x-chunks
of ok live on the four 32-partition bands and run as concurrent row-group
matmuls on the PE's independent 32x32 sub-arrays (tile_position row tiling).
"""

import numpy as np

import concourse.bass as bass
import concourse.mybir as mybir
import concourse.tile as tile
from concourse import bacc
from concourse.bass_utils import run_bass_kernel_spmd

F32 = mybir.dt.float32
NCORES = 8
B, H, W, D = 32, 64, 64, 64
AK_COUNT = float(B * H * 1024)  # 2_097_152 elements in ak

LAST_RESULTS = None   # test harness introspection


def _build_kernel():
    nc = bacc.Bacc("TRN2", target_bir_lowering=False, num_devices=NCORES)

    # ---- I/O ------------------------------------------------------------
    # okB: ok rearranged host-side to partitions p = xh*32 + b (xh = x//8),
    #      free = (x%8)*1024 + y*32 + z.  Band xh is the matmul rhs for
    #      corr columns [xh*8192, (xh+1)*8192) and the source of oksum.
    okB = nc.dram_tensor("okB", [128, 8192], F32, kind="ExternalInput")
    # fkT: this core's fk slab, partitions p = b*4 + (h_loc//2),
    #      free = (h_loc%2)*4096 + w*64 + d.
    fkT = nc.dram_tensor("fkT", [128, 8192], F32, kind="ExternalInput")
    # selector constants (see host side)
    sel4r = nc.dram_tensor("sel4r", [4, 128, 128], F32, kind="ExternalInput")
    selb = nc.dram_tensor("selb", [128, 32], F32, kind="ExternalInput")
    ident = nc.dram_tensor("ident", [128, 128], F32, kind="ExternalInput")

    # corr slab, rows = h_loc*64 + w, cols = xyz
    corr_out = nc.dram_tensor("corr_out", [512, 32768], F32, kind="ExternalOutput")
    # ak_hat slab, rows = h_loc*32 + q  (== b2_loc*64 + h2), cols = y*32+z
    akhat_out = nc.dram_tensor("akhat_out", [256, 1024], F32, kind="ExternalOutput")

    with tile.TileContext(nc) as tc:
        with (
            tc.tile_pool(name="okio", bufs=1) as okio,
            tc.tile_pool(name="consts", bufs=1) as consts,
            tc.tile_pool(name="small", bufs=1) as small,
            tc.tile_pool(name="cps", bufs=6, space="PSUM") as cps,
            tc.tile_pool(name="sps", bufs=2, space="PSUM") as sps,
            tc.tile_pool(name="dram", bufs=1, space="DRAM") as dram_pool,
        ):
            # ---- Phase A: loads (fkT first -- it gates fsum_rep, which
            # gates the whole corr loop; okB band 0 arrives during reduce)
            with tc.tile_pool(name="fkio", bufs=1) as fkio:
                fkT_sb = fkio.tile([128, 8192], F32, name="fkT_sb")
                nc.sync.dma_start(fkT_sb[:], fkT[:])
                okB_sb = okio.tile([128, 8192], F32, name="okB_sb")
                nc.sync.dma_start(okB_sb[:], okB[:])

                sel_sb = consts.tile([128, 4, 128], F32, name="sel_sb")
                nc.sync.dma_start(sel_sb[:], sel4r.rearrange("h p m -> p h m"))
                selb_sb = consts.tile([128, 32], F32, name="selb_sb")
                nc.sync.dma_start(selb_sb[:], selb[:])
                ident_sb = consts.tile([128, 128], F32, name="ident_sb")
                nc.sync.dma_start(ident_sb[:], ident[:])

                # ---- Phase B: fsum / oksum -----------------------------
                # fsumT[p=(b,hh), g=(hl,w)] = sum_d fkT[p, g*64 + d]
                fsumT = small.tile([128, 128], F32, name="fsumT")
                nc.vector.tensor_reduce(
                    fsumT[:],
                    fkT_sb.rearrange("p (g d) -> p g d", d=64),
                    axis=mybir.AxisListType.X,
                    op=mybir.AluOpType.add,
                )
            # fkT SBUF space released here (pool closed)

            # fsum_rep[(xh', b), hloc*64 + w] = fsum[b, hloc, w] for every xh'
            fsum_rep = small.tile([128, 512], F32, name="fsum_rep")
            for hh in range(4):
                fsr_ps = sps.tile([128, 512], F32, name="sps_t", tag="sps")
                nc.tensor.matmul(
                    fsr_ps[:, 0:128],
                    sel_sb[:, hh, :],
                    fsumT[:],
                    start=True,
                    stop=True,
                )
                nc.any.tensor_copy(
                    out=fsum_rep[:, hh * 128 : (hh + 1) * 128], in_=fsr_ps[:, 0:128]
                )

            # okS1[p=(xh,b), k] = sum_xl okB[p, xl*1024 + k]
            okS1 = small.tile([128, 1024], F32, name="okS1")
            nc.vector.tensor_reduce(
                okS1[:],
                okB_sb.rearrange("p (xl k) -> p k xl", xl=8),
                axis=mybir.AxisListType.X,
                op=mybir.AluOpType.add,
            )
            # oksum[b, k] = sum_x ok[b, x, yz]   (32 partitions)
            oksum_sb = small.tile([32, 1024], F32, name="oksum_sb")
            for j in range(2):
                okm_ps = sps.tile([128, 512], F32, name="sps_t", tag="sps")
                nc.tensor.matmul(
                    okm_ps[0:32, :],
                    selb_sb[:],
                    okS1[:, j * 512 : (j + 1) * 512],
                    start=True,
                    stop=True,
                )
                nc.any.tensor_copy(
                    out=oksum_sb[:, j * 512 : (j + 1) * 512], in_=okm_ps[0:32, :]
                )

            # ---- Phase C: global mean via one scalar AllReduce ----------
            # fpair[b, hloc*32+q] = fsum[b,hloc,2q] + fsum[b,hloc,2q+1]
            fpair_sb = small.tile([32, 256], F32, name="fpair_sb")
            fsum_pairs = fsum_rep[0:32].rearrange("p (hq two) -> p hq two", two=2)
            nc.vector.tensor_add(
                fpair_sb[:], fsum_pairs[:, :, 0], fsum_pairs[:, :, 1]
            )

            okt = small.tile([32, 1], F32, name="okt")
            nc.vector.tensor_reduce(
                okt[:], oksum_sb[:], axis=mybir.AxisListType.X, op=mybir.AluOpType.add
            )
            fst = small.tile([32, 1], F32, name="fst")
            nc.vector.tensor_reduce(
                fst[:], fsum_rep[0:32], axis=mybir.AxisListType.X,
                op=mybir.AluOpType.add,
            )
            prod = small.tile([32, 1], F32, name="prod")
            nc.vector.tensor_mul(prod[:], okt[:], fst[:])

            ones32 = small.tile([32, 1], F32, name="ones32")
            nc.vector.memset(ones32[:], 1.0)
            pi_ps = sps.tile([128, 512], F32, name="sps_t", tag="sps")
            nc.tensor.matmul(pi_ps[0:1, 0:1], prod[:], ones32[:], start=True, stop=True)
            pi_sb = small.tile([1, 1], F32, name="pi_sb")
            nc.any.tensor_copy(out=pi_sb[:], in_=pi_ps[0:1, 0:1])

            cc_in = dram_pool.tile([1, 1], F32, name="cc_in")
            cc_out = dram_pool.tile([1, 1], F32, name="cc_out")
            nc.sync.dma_start(cc_in[:], pi_sb[:])
            nc.gpsimd.collective_compute(
                "AllReduce",
                mybir.AluOpType.add,
                replica_groups=[list(range(NCORES))],
                ins=[cc_in.opt()],
                outs=[cc_out.opt()],
            )
            tot_sb = small.tile([1, 1], F32, name="tot_sb")
            nc.sync.dma_start(tot_sb[:], cc_out[:])

            avg1 = small.tile([1, 1], F32, name="avg1")
            nc.vector.tensor_scalar_mul(avg1[:], tot_sb[:], 1.0 / AK_COUNT)
            ones_r = small.tile([1, 128], F32, name="ones_r")
            nc.vector.memset(ones_r[:], 1.0)
            avgb_ps = sps.tile([128, 512], F32, name="sps_t", tag="sps")
            nc.tensor.matmul(
                avgb_ps[:, 0:1], ones_r[:], avg1[:], start=True, stop=True
            )
            avg_bc = small.tile([128, 1], F32, name="avg_bc")
            nc.any.tensor_copy(out=avg_bc[:], in_=avgb_ps[:, 0:1])

            # ---- ak matmuls early: akT[k, row] needs only oksum/fpair ---
            # One SBUF tile (128, 8, 256): chunk kc on the middle axis.
            akT = small.tile([128, 8, 256], F32, name="akT")
            for kc in range(8):
                akT_ps = sps.tile([128, 512], F32, name="sps_t", tag="sps")
                nc.tensor.matmul(
                    akT_ps[:, 0:256],
                    oksum_sb[:, kc * 128 : (kc + 1) * 128],
                    fpair_sb[:],
                    start=True,
                    stop=True,
                )
                nc.vector.tensor_copy(out=akT[:, kc, :], in_=akT_ps[:, 0:256])

            # ---- Phase D: corr slab -- 4 row bands streamed concurrently
            # loop (m, s, j) with xh innermost: 4 consecutive matmuls hit
            # disjoint 32-row groups of the PE array and overlap.
            with tc.tile_pool(name="stage", bufs=6) as stage_pool:
                for m in range(4):
                    for s in range(2):
                        stgs = [
                            stage_pool.tile([128, 4096], F32, name="stg", tag="stg")
                            for _ in range(4)
                        ]
                        for j in range(8):
                            ns = s * 8 + j
                            for xh in range(4):
                                pbase = 32 * xh
                                cp = cps.tile([128, 512], F32, name="cp", tag="cp")
                                nc.tensor.matmul(
                                    cp[:],
                                    fsum_rep[
                                        pbase : pbase + 32, m * 128 : (m + 1) * 128
                                    ],
                                    okB_sb[
                                        pbase : pbase + 32,
                                        ns * 512 : (ns + 1) * 512,
                                    ],
                                    start=True,
                                    stop=True,
                                    tile_position=(pbase, 0),
                                )
                                if (j + xh) % 2 == 0:
                                    nc.scalar.copy(
                                        out=stgs[xh][:, j * 512 : (j + 1) * 512],
                                        in_=cp[:],
                                    )
                                else:
                                    nc.vector.tensor_copy(
                                        out=stgs[xh][:, j * 512 : (j + 1) * 512],
                                        in_=cp[:],
                                    )
                        for xh in range(4):
                            nc.sync.dma_start(
                                corr_out[
                                    m * 128 : (m + 1) * 128,
                                    xh * 8192 + s * 4096 : xh * 8192 + (s + 1) * 4096,
                                ],
                                stgs[xh][:],
                            )

                # ---- Phase E: ak_hat stats + normalize (batched) --------
                # mask/scale akT against avg, segmented min/max over h2
                mask = small.tile([128, 8, 256], F32, name="mask")
                nc.vector.tensor_scalar(
                    mask[:], akT[:], avg_bc[:], None, op0=mybir.AluOpType.is_gt
                )
                nc.vector.tensor_mul(akT[:], akT[:], mask[:])

                grp = akT.rearrange("p c (g h) -> p (c g) h", h=64)
                mins = small.tile([128, 32], F32, name="mins")
                nc.vector.tensor_reduce(
                    mins[:], grp, axis=mybir.AxisListType.X, op=mybir.AluOpType.min
                )
                for cg in range(32):
                    kc, g = divmod(cg, 4)
                    nc.vector.tensor_scalar(
                        akT[:, kc, g * 64 : (g + 1) * 64],
                        akT[:, kc, g * 64 : (g + 1) * 64],
                        mins[:, cg : cg + 1],
                        None,
                        op0=mybir.AluOpType.subtract,
                    )
                maxs = small.tile([128, 32], F32, name="maxs")
                nc.vector.tensor_reduce(
                    maxs[:], grp, axis=mybir.AxisListType.X, op=mybir.AluOpType.max
                )
                rec = small.tile([128, 32], F32, name="rec")
                nc.vector.reciprocal(rec[:], maxs[:])
                for cg in range(32):
                    kc, g = divmod(cg, 4)
                    nc.vector.tensor_scalar(
                        akT[:, kc, g * 64 : (g + 1) * 64],
                        akT[:, kc, g * 64 : (g + 1) * 64],
                        rec[:, cg : cg + 1],
                        None,
                        op0=mybir.AluOpType.mult,
                    )

                # transpose (k x row) -> (row x k), two 128x128 blocks per kc
                akh_r0 = small.tile([128, 1024], F32, name="akh_r0")
                akh_r1 = small.tile([128, 1024], F32, name="akh_r1")
                for kc in range(8):
                    for half, dst in ((0, akh_r0), (1, akh_r1)):
                        t_ps = sps.tile([128, 512], F32, name="sps_t", tag="sps")
                        nc.tensor.transpose(
                            t_ps[:, 0:128],
                            akT[:, kc, half * 128 : (half + 1) * 128],
                            ident_sb[:],
                        )
                        nc.any.tensor_copy(
                            out=dst[:, kc * 128 : (kc + 1) * 128], in_=t_ps[:, 0:128]
                        )

                nc.sync.dma_start(akhat_out[0:128, :], akh_r0[:])
                nc.sync.dma_start(akhat_out[128:256, :], akh_r1[:])

    nc.compile()
    return nc


_CACHE = {}


def _get_nc():
    if "nc" not in _CACHE:
        _CACHE["nc"] = _build_kernel()
    return _CACHE["nc"]


def _make_consts():
    sel4r = np.zeros((4, 128, 128), dtype=np.float32)
    b = np.arange(32)
    for hh in range(4):
        for x in range(4):
            sel4r[hh, b * 4 + hh, x * 32 + b] = 1.0
    selb = np.tile(np.eye(32, dtype=np.float32), (4, 1))
    ident = np.eye(128, dtype=np.float32)
    return sel4r, selb, ident


def kernel(ok, fk):
    global LAST_RESULTS
    import os

    ok = np.ascontiguousarray(np.asarray(ok), dtype=np.float32)
    fk = np.ascontiguousarray(np.asarray(fk), dtype=np.float32)
    assert ok.shape == (B, 32, 32, 32) and fk.shape == (B, H, W, D)

    okB = np.ascontiguousarray(
        ok.reshape(32, 4, 8192).transpose(1, 0, 2).reshape(128, 8192)
    )
    sel4r, selb, ident = _make_consts()

    in_maps = []
    for i in range(NCORES):
        fkT = np.ascontiguousarray(fk[:, 8 * i : 8 * (i + 1)]).reshape(128, 8192)
        in_maps.append(
            {"okB": okB, "fkT": fkT, "sel4r": sel4r, "selb": selb, "ident": ident}
        )

    nc = _get_nc()
    res = run_bass_kernel_spmd(
        nc,
        in_maps,
        list(range(NCORES)),
        trace=bool(int(os.environ.get("KERNEL_TRACE", "0"))),
    )
    LAST_RESULTS = res

    corr = np.empty((64, 64 * 32768), dtype=np.float32)
    akh = np.empty((32, 64, 1024), dtype=np.float32)
    for i in range(NCORES):
        r = res.results[i]
        corr[8 * i : 8 * (i + 1)] = r["corr_out"].reshape(8, 64 * 32768)
        akh[4 * i : 4 * (i + 1)] = r["akhat_out"].reshape(4, 64, 1024)
    corr = corr.reshape(B, H, W, 1024)
    return corr, akh


# revision 4
# speedup vs baseline: 1.8575x; 1.0058x over previous
"""Trainium2 Bass kernel for nn_CorrelationBlock.

Reference computation (B=32, H=64, W=64, D=64, X=Y=Z=32):
    fsum = fk.sum(-1)                                  # (B, H, W)
    corr = einsum('bxyz,bhw->hwxyz', ok, fsum)         # (H, W, X, Y, Z)
    corr = corr.reshape(B, H, W, 1024)                 # flat reinterpret
    ak   = corr.sum(axis=2)                            # (B, H, 1024)
    avg  = ak.mean()
    akh  = where(ak > avg, ak, 0)
    akh  = akh - akh.min(axis=1, keepdims=True)
    akh  = akh / akh.max(axis=1, keepdims=True)
    returns (corr, akh)

Sharding: H is split into 8 slabs of 8; core i computes corr rows
h in [8i, 8i+8) -- exactly corr.reshape(...)[4i:4i+4] -- plus the matching
ak rows.  ak factorizes without touching the 512MB corr tensor:
    ak[b2,h2,yz] = sum_b (fsum[b,h,2q] + fsum[b,h,2q+1]) * (sum_x ok[b,x,yz])
with h = 2*b2 + h2//32, q = h2%32.  The global mean over ak reduces to
sum_b (sum_hw fsum) * (sum_xyz ok), accumulated with one scalar AllReduce.

The corr matmul has contraction K=32 (the batch dim), so the four x-chunks
of ok live on the four 32-partition bands and run as concurrent row-group
matmuls on the PE's independent 32x32 sub-arrays (tile_position row tiling).
Side computations (oksum / ak / mean / normalize) are emitted between corr
m-groups so they fill PE/DVE slack without stalling the output DMA stream.
"""

import numpy as np

import concourse.bass as bass
import concourse.mybir as mybir
import concourse.tile as tile
from concourse import bacc
from concourse.bass_utils import run_bass_kernel_spmd

F32 = mybir.dt.float32
NCORES = 8
B, H, W, D = 32, 64, 64, 64
AK_COUNT = float(B * H * 1024)  # 2_097_152 elements in ak

LAST_RESULTS = None   # test harness introspection


def _build_kernel():
    nc = bacc.Bacc("TRN2", target_bir_lowering=False, num_devices=NCORES)

    okB = nc.dram_tensor("okB", [128, 8192], F32, kind="ExternalInput")
    fkT = nc.dram_tensor("fkT", [128, 8192], F32, kind="ExternalInput")
    sel4r = nc.dram_tensor("sel4r", [4, 128, 128], F32, kind="ExternalInput")
    selb = nc.dram_tensor("selb", [128, 32], F32, kind="ExternalInput")
    ident = nc.dram_tensor("ident", [128, 128], F32, kind="ExternalInput")

    corr_out = nc.dram_tensor("corr_out", [512, 32768], F32, kind="ExternalOutput")
    akhat_out = nc.dram_tensor("akhat_out", [256, 1024], F32, kind="ExternalOutput")

    with tile.TileContext(nc) as tc:
        with (
            tc.tile_pool(name="okio", bufs=1) as okio,
            tc.tile_pool(name="consts", bufs=1) as consts,
            tc.tile_pool(name="small", bufs=1) as small,
            tc.tile_pool(name="cps", bufs=6, space="PSUM") as cps,
            tc.tile_pool(name="sps", bufs=2, space="PSUM") as sps,
            tc.tile_pool(name="dram", bufs=1, space="DRAM") as dram_pool,
        ):
            okB_sb = okio.tile([128, 8192], F32, name="okB_sb")
            fsum_rep = small.tile([128, 512], F32, name="fsum_rep")
            fsumT = small.tile([128, 128], F32, name="fsumT")

            # ---- Phase A/B: fkT chunk-loaded + chunk-reduced so fsum_rep
            # (which gates the corr loop) is ready right after the load.
            with tc.tile_pool(name="fkio", bufs=2) as fkio:
                for c in range(4):
                    fk_ch = fkio.tile([128, 2048], F32, name="fk_ch", tag="fk")
                    nc.sync.dma_start(fk_ch[:], fkT[:, c * 2048 : (c + 1) * 2048])
                    nc.vector.tensor_reduce(
                        fsumT[:, c * 32 : (c + 1) * 32],
                        fk_ch.rearrange("p (g d) -> p g d", d=64),
                        axis=mybir.AxisListType.X,
                        op=mybir.AluOpType.add,
                    )
                nc.sync.dma_start(okB_sb[:], okB[:])

                sel_sb = consts.tile([128, 4, 128], F32, name="sel_sb")
                nc.sync.dma_start(sel_sb[:], sel4r.rearrange("h p m -> p h m"))
                selb_sb = consts.tile([128, 32], F32, name="selb_sb")
                nc.sync.dma_start(selb_sb[:], selb[:])
                ident_sb = consts.tile([128, 128], F32, name="ident_sb")
                nc.sync.dma_start(ident_sb[:], ident[:])

                # fsum_rep[(xh', b), hloc*64+w] = fsum[b, hloc, w] per band
                for hh in range(4):
                    fsr_ps = sps.tile([128, 512], F32, name="sps_t", tag="sps")
                    nc.tensor.matmul(
                        fsr_ps[:, 0:128],
                        sel_sb[:, hh, :],
                        fsumT[:],
                        start=True,
                        stop=True,
                    )
                    nc.any.tensor_copy(
                        out=fsum_rep[:, hh * 128 : (hh + 1) * 128],
                        in_=fsr_ps[:, 0:128],
                    )

            # ---- corr loop pieces -------------------------------------
            with tc.tile_pool(name="stage", bufs=8) as stage_pool:

                def corr_group(m, s, split):
                    """matmul+evac+DMA for output rows m*128.. cols of
                    half s; xh innermost so the 4 row-bands overlap."""
                    halves = 2 if split else 1
                    width = 4096 // halves
                    for h2 in range(halves):
                        stgs = [
                            stage_pool.tile([128, width], F32, name="stg", tag="stg")
                            for _ in range(4)
                        ]
                        for j in range(8 // halves):
                            ns = s * 8 + h2 * (8 // halves) + j
                            for xh in range(4):
                                pbase = 32 * xh
                                cp = cps.tile([128, 512], F32, name="cp", tag="cp")
                                nc.tensor.matmul(
                                    cp[:],
                                    fsum_rep[
                                        pbase : pbase + 32, m * 128 : (m + 1) * 128
                                    ],
                                    okB_sb[
                                        pbase : pbase + 32, ns * 512 : (ns + 1) * 512
                                    ],
                                    start=True,
                                    stop=True,
                                    tile_position=(pbase, 0),
                                )
                                if (j + xh) % 2 == 0:
                                    nc.scalar.copy(
                                        out=stgs[xh][:, j * 512 : (j + 1) * 512],
                                        in_=cp[:],
                                    )
                                else:
                                    nc.vector.tensor_copy(
                                        out=stgs[xh][:, j * 512 : (j + 1) * 512],
                                        in_=cp[:],
                                    )
                        for xh in range(4):
                            col0 = xh * 8192 + s * 4096 + h2 * width
                            nc.sync.dma_start(
                                corr_out[m * 128 : (m + 1) * 128, col0 : col0 + width],
                                stgs[xh][:],
                            )

                # m=0 with split stages so the first output DMA fires early
                corr_group(0, 0, split=True)
                corr_group(0, 1, split=False)

                # ---- side chain part 1: oksum, mean partial, collective,
                # ak matmuls (fills PE/DVE slack behind m=0's DMA drain)
                okS1 = small.tile([128, 1024], F32, name="okS1")
                nc.vector.tensor_reduce(
                    okS1[:],
                    okB_sb.rearrange("p (xl k) -> p k xl", xl=8),
                    axis=mybir.AxisListType.X,
                    op=mybir.AluOpType.add,
                )
                oksum_sb = small.tile([32, 1024], F32, name="oksum_sb")
                for j in range(2):
                    okm_ps = sps.tile([128, 512], F32, name="sps_t", tag="sps")
                    nc.tensor.matmul(
                        okm_ps[0:32, :],
                        selb_sb[:],
                        okS1[:, j * 512 : (j + 1) * 512],
                        start=True,
                        stop=True,
                    )
                    nc.any.tensor_copy(
                        out=oksum_sb[:, j * 512 : (j + 1) * 512], in_=okm_ps[0:32, :]
                    )

                fpair_sb = small.tile([32, 256], F32, name="fpair_sb")
                fsum_pairs = fsum_rep[0:32].rearrange(
                    "p (hq two) -> p hq two", two=2
                )
                nc.vector.tensor_add(
                    fpair_sb[:], fsum_pairs[:, :, 0], fsum_pairs[:, :, 1]
                )

                okt = small.tile([32, 1], F32, name="okt")
                nc.vector.tensor_reduce(
                    okt[:], oksum_sb[:], axis=mybir.AxisListType.X,
                    op=mybir.AluOpType.add,
                )
                fst = small.tile([32, 1], F32, name="fst")
                nc.vector.tensor_reduce(
                    fst[:], fsum_rep[0:32], axis=mybir.AxisListType.X,
                    op=mybir.AluOpType.add,
                )
                prod = small.tile([32, 1], F32, name="prod")
                nc.vector.tensor_mul(prod[:], okt[:], fst[:])
                ones32 = small.tile([32, 1], F32, name="ones32")
                nc.vector.memset(ones32[:], 1.0)
                pi_ps = sps.tile([128, 512], F32, name="sps_t", tag="sps")
                nc.tensor.matmul(
                    pi_ps[0:1, 0:1], prod[:], ones32[:], start=True, stop=True
                )
                pi_sb = small.tile([1, 1], F32, name="pi_sb")
                nc.any.tensor_copy(out=pi_sb[:], in_=pi_ps[0:1, 0:1])

                cc_in = dram_pool.tile([1, 1], F32, name="cc_in")
                cc_out = dram_pool.tile([1, 1], F32, name="cc_out")
                nc.sync.dma_start(cc_in[:], pi_sb[:])
                nc.gpsimd.collective_compute(
                    "AllReduce",
                    mybir.AluOpType.add,
                    replica_groups=[list(range(NCORES))],
                    ins=[cc_in.opt()],
                    outs=[cc_out.opt()],
                )

                akT = small.tile([128, 8, 256], F32, name="akT")
                for kc in range(8):
                    akT_ps = sps.tile([128, 512], F32, name="sps_t", tag="sps")
                    nc.tensor.matmul(
                        akT_ps[:, 0:256],
                        oksum_sb[:, kc * 128 : (kc + 1) * 128],
                        fpair_sb[:],
                        start=True,
                        stop=True,
                    )
                    nc.vector.tensor_copy(out=akT[:, kc, :], in_=akT_ps[:, 0:256])

                for s in range(2):
                    corr_group(1, s, split=False)

                # ---- side chain part 2: collective result -> avg_bc ----
                tot_sb = small.tile([1, 1], F32, name="tot_sb")
                nc.sync.dma_start(tot_sb[:], cc_out[:])
                avg1 = small.tile([1, 1], F32, name="avg1")
                nc.vector.tensor_scalar_mul(avg1[:], tot_sb[:], 1.0 / AK_COUNT)
                ones_r = small.tile([1, 128], F32, name="ones_r")
                nc.vector.memset(ones_r[:], 1.0)
                avgb_ps = sps.tile([128, 512], F32, name="sps_t", tag="sps")
                nc.tensor.matmul(
                    avgb_ps[:, 0:1], ones_r[:], avg1[:], start=True, stop=True
                )
                avg_bc = small.tile([128, 1], F32, name="avg_bc")
                nc.any.tensor_copy(out=avg_bc[:], in_=avgb_ps[:, 0:1])

                for s in range(2):
                    corr_group(2, s, split=False)

                # ---- side chain part 3: mask/min/max/normalize + output
                mask = small.tile([128, 8, 256], F32, name="mask")
                nc.vector.tensor_scalar(
                    mask[:], akT[:], avg_bc[:], None, op0=mybir.AluOpType.is_gt
                )
                nc.vector.tensor_mul(akT[:], akT[:], mask[:])

                grp = akT.rearrange("p c (g h) -> p (c g) h", h=64)
                mins = small.tile([128, 32], F32, name="mins")
                nc.vector.tensor_reduce(
                    mins[:], grp, axis=mybir.AxisListType.X, op=mybir.AluOpType.min
                )
                for cg in range(32):
                    kc, g = divmod(cg, 4)
                    nc.vector.tensor_scalar(
                        akT[:, kc, g * 64 : (g + 1) * 64],
                        akT[:, kc, g * 64 : (g + 1) * 64],
                        mins[:, cg : cg + 1],
                        None,
                        op0=mybir.AluOpType.subtract,
                    )
                maxs = small.tile([128, 32], F32, name="maxs")
                nc.vector.tensor_reduce(
                    maxs[:], grp, axis=mybir.AxisListType.X, op=mybir.AluOpType.max
                )
                rec = small.tile([128, 32], F32, name="rec")
                nc.vector.reciprocal(rec[:], maxs[:])
                for cg in range(32):
                    kc, g = divmod(cg, 4)
                    nc.vector.tensor_scalar(
                        akT[:, kc, g * 64 : (g + 1) * 64],
                        akT[:, kc, g * 64 : (g + 1) * 64],
                        rec[:, cg : cg + 1],
                        None,
                        op0=mybir.AluOpType.mult,
                    )

                akh_r0 = small.tile([128, 1024], F32, name="akh_r0")
                akh_r1 = small.tile([128, 1024], F32, name="akh_r1")
                for kc in range(8):
                    for half, dst in ((0, akh_r0), (1, akh_r1)):
                        t_ps = sps.tile([128, 512], F32, name="sps_t", tag="sps")
                        nc.tensor.transpose(
                            t_ps[:, 0:128],
                            akT[:, kc, half * 128 : (half + 1) * 128],
                            ident_sb[:],
                        )
                        nc.any.tensor_copy(
                            out=dst[:, kc * 128 : (kc + 1) * 128], in_=t_ps[:, 0:128]
                        )
                nc.sync.dma_start(akhat_out[0:128, :], akh_r0[:])
                nc.sync.dma_start(akhat_out[128:256, :], akh_r1[:])

                for s in range(2):
                    corr_group(3, s, split=False)

    nc.compile()
    return nc


_CACHE = {}


def _get_nc():
    if "nc" not in _CACHE:
        _CACHE["nc"] = _build_kernel()
    return _CACHE["nc"]


def _make_consts():
    sel4r = np.zeros((4, 128, 128), dtype=np.float32)
    b = np.arange(32)
    for hh in range(4):
        for x in range(4):
            sel4r[hh, b * 4 + hh, x * 32 + b] = 1.0
    selb = np.tile(np.eye(32, dtype=np.float32), (4, 1))
    ident = np.eye(128, dtype=np.float32)
    return sel4r, selb, ident


def kernel(ok, fk):
    global LAST_RESULTS
    import os

    ok = np.ascontiguousarray(np.asarray(ok), dtype=np.float32)
    fk = np.ascontiguousarray(np.asarray(fk), dtype=np.float32)
    assert ok.shape == (B, 32, 32, 32) and fk.shape == (B, H, W, D)

    okB = np.ascontiguousarray(
        ok.reshape(32, 4, 8192).transpose(1, 0, 2).reshape(128, 8192)
    )
    sel4r, selb, ident = _make_consts()

    in_maps = []
    for i in range(NCORES):
        fkT = np.ascontiguousarray(fk[:, 8 * i : 8 * (i + 1)]).reshape(128, 8192)
        in_maps.append(
            {"okB": okB, "fkT": fkT, "sel4r": sel4r, "selb": selb, "ident": ident}
        )

    nc = _get_nc()
    res = run_bass_kernel_spmd(
        nc,
        in_maps,
        list(range(NCORES)),
        trace=bool(int(os.environ.get("KERNEL_TRACE", "0"))),
    )
    LAST_RESULTS = res

    corr = np.empty((64, 64 * 32768), dtype=np.float32)
    akh = np.empty((32, 64, 1024), dtype=np.float32)
    for i in range(NCORES):
        r = res.results[i]
        corr[8 * i : 8 * (i + 1)] = r["corr_out"].reshape(8, 64 * 32768)
        akh[4 * i : 4 * (i + 1)] = r["akhat_out"].reshape(4, 64, 1024)
    corr = corr.reshape(B, H, W, 1024)
    return corr, akh


# revision 5
# speedup vs baseline: 1.9036x; 1.0249x over previous
"""Trainium2 Bass kernel for nn_CorrelationBlock.

Reference computation (B=32, H=64, W=64, D=64, X=Y=Z=32):
    fsum = fk.sum(-1)                                  # (B, H, W)
    corr = einsum('bxyz,bhw->hwxyz', ok, fsum)         # (H, W, X, Y, Z)
    corr = corr.reshape(B, H, W, 1024)                 # flat reinterpret
    ak   = corr.sum(axis=2)                            # (B, H, 1024)
    avg  = ak.mean()
    akh  = where(ak > avg, ak, 0)
    akh  = akh - akh.min(axis=1, keepdims=True)
    akh  = akh / akh.max(axis=1, keepdims=True)
    returns (corr, akh)

Sharding: H is split into 8 slabs of 8; core i computes corr rows
h in [8i, 8i+8) -- exactly corr.reshape(...)[4i:4i+4] -- plus the matching
ak rows.  ak factorizes without touching the 512MB corr tensor:
    ak[b2,h2,yz] = sum_b (fsum[b,h,2q] + fsum[b,h,2q+1]) * (sum_x ok[b,x,yz])
with h = 2*b2 + h2//32, q = h2%32.  The global mean over ak reduces to
sum_b (sum_hw fsum) * (sum_xyz ok), accumulated with one scalar AllReduce.

The corr matmul has contraction K=32 (the batch dim), so the four x-chunks
of ok live on the four 32-partition bands and run as concurrent row-group
matmuls on the PE's independent 32x32 sub-arrays (tile_position row tiling).
Side computations (oksum / ak / mean / normalize) are emitted between corr
m-groups so they fill PE/DVE slack without stalling the output DMA stream.
"""

import numpy as np

import concourse.bass as bass
import concourse.mybir as mybir
import concourse.tile as tile
from concourse import bacc
from concourse.bass_utils import run_bass_kernel_spmd

F32 = mybir.dt.float32
NCORES = 8
B, H, W, D = 32, 64, 64, 64
AK_COUNT = float(B * H * 1024)  # 2_097_152 elements in ak

LAST_RESULTS = None   # test harness introspection


def _build_kernel():
    nc = bacc.Bacc("TRN2", target_bir_lowering=False, num_devices=NCORES)

    okB = nc.dram_tensor("okB", [128, 8192], F32, kind="ExternalInput")
    fkT = nc.dram_tensor("fkT", [128, 8192], F32, kind="ExternalInput")
    sel4r = nc.dram_tensor("sel4r", [4, 128, 128], F32, kind="ExternalInput")
    selb = nc.dram_tensor("selb", [128, 32], F32, kind="ExternalInput")
    ident = nc.dram_tensor("ident", [128, 128], F32, kind="ExternalInput")

    corr_out = nc.dram_tensor("corr_out", [512, 32768], F32, kind="ExternalOutput")
    akhat_out = nc.dram_tensor("akhat_out", [256, 1024], F32, kind="ExternalOutput")

    with tile.TileContext(nc) as tc:
        with (
            tc.tile_pool(name="okio", bufs=1) as okio,
            tc.tile_pool(name="consts", bufs=1) as consts,
            tc.tile_pool(name="small", bufs=1) as small,
            tc.tile_pool(name="cps", bufs=6, space="PSUM") as cps,
            tc.tile_pool(name="sps", bufs=2, space="PSUM") as sps,
            tc.tile_pool(name="dram", bufs=1, space="DRAM") as dram_pool,
        ):
            okB_sb = okio.tile([128, 8192], F32, name="okB_sb")
            fsum_rep = small.tile([128, 512], F32, name="fsum_rep")
            fsumT = small.tile([128, 128], F32, name="fsumT")

            # ---- Phase A/B: fkT chunk-loaded + chunk-reduced so fsum_rep
            # (which gates the corr loop) is ready right after the load.
            with tc.tile_pool(name="fkio", bufs=4) as fkio:
                sel_sb = consts.tile([128, 4, 128], F32, name="sel_sb")
                nc.sync.dma_start(sel_sb[:], sel4r.rearrange("h p m -> p h m"))
                selb_sb = consts.tile([128, 32], F32, name="selb_sb")
                nc.sync.dma_start(selb_sb[:], selb[:])
                ident_sb = consts.tile([128, 128], F32, name="ident_sb")
                nc.sync.dma_start(ident_sb[:], ident[:])

                for c in range(4):
                    fk_ch = fkio.tile([128, 2048], F32, name="fk_ch", tag="fk")
                    nc.sync.dma_start(fk_ch[:], fkT[:, c * 2048 : (c + 1) * 2048])
                    nc.vector.tensor_reduce(
                        fsumT[:, c * 32 : (c + 1) * 32],
                        fk_ch.rearrange("p (g d) -> p g d", d=64),
                        axis=mybir.AxisListType.X,
                        op=mybir.AluOpType.add,
                    )
                nc.sync.dma_start(okB_sb[:], okB[:])

                # fsum_rep[(xh', b), hloc*64+w] = fsum[b, hloc, w] per band
                for hh in range(4):
                    fsr_ps = sps.tile([128, 512], F32, name="sps_t", tag="sps")
                    nc.tensor.matmul(
                        fsr_ps[:, 0:128],
                        sel_sb[:, hh, :],
                        fsumT[:],
                        start=True,
                        stop=True,
                    )
                    nc.any.tensor_copy(
                        out=fsum_rep[:, hh * 128 : (hh + 1) * 128],
                        in_=fsr_ps[:, 0:128],
                    )

            # ---- corr loop pieces -------------------------------------
            with tc.tile_pool(name="stage", bufs=8) as stage_pool:

                def corr_group(m, s, split):
                    """matmul+evac+DMA for output rows m*128.. cols of
                    half s; xh innermost so the 4 row-bands overlap."""
                    halves = 2 if split else 1
                    width = 4096 // halves
                    for h2 in range(halves):
                        stgs = [
                            stage_pool.tile([128, width], F32, name="stg", tag="stg")
                            for _ in range(4)
                        ]
                        for j in range(8 // halves):
                            ns = s * 8 + h2 * (8 // halves) + j
                            for xh in range(4):
                                pbase = 32 * xh
                                cp = cps.tile([128, 512], F32, name="cp", tag="cp")
                                nc.tensor.matmul(
                                    cp[:],
                                    fsum_rep[
                                        pbase : pbase + 32, m * 128 : (m + 1) * 128
                                    ],
                                    okB_sb[
                                        pbase : pbase + 32, ns * 512 : (ns + 1) * 512
                                    ],
                                    start=True,
                                    stop=True,
                                    tile_position=(pbase, 0),
                                )
                                if (j + xh) % 4 != 1:
                                    nc.scalar.copy(
                                        out=stgs[xh][:, j * 512 : (j + 1) * 512],
                                        in_=cp[:],
                                    )
                                else:
                                    nc.vector.tensor_copy(
                                        out=stgs[xh][:, j * 512 : (j + 1) * 512],
                                        in_=cp[:],
                                    )
                        for xh in range(4):
                            col0 = xh * 8192 + s * 4096 + h2 * width
                            nc.sync.dma_start(
                                corr_out[m * 128 : (m + 1) * 128, col0 : col0 + width],
                                stgs[xh][:],
                            )

                # m=0 with split stages so the first output DMA fires early
                corr_group(0, 0, split=True)
                corr_group(0, 1, split=False)

                # ---- side chain part 1: oksum, mean partial, collective,
                # ak matmuls (fills PE/DVE slack behind m=0's DMA drain)
                okS1 = small.tile([128, 1024], F32, name="okS1")
                nc.vector.tensor_reduce(
                    okS1[:],
                    okB_sb.rearrange("p (xl k) -> p k xl", xl=8),
                    axis=mybir.AxisListType.X,
                    op=mybir.AluOpType.add,
                )
                oksum_sb = small.tile([32, 1024], F32, name="oksum_sb")
                for j in range(2):
                    okm_ps = sps.tile([128, 512], F32, name="sps_t", tag="sps")
                    nc.tensor.matmul(
                        okm_ps[0:32, :],
                        selb_sb[:],
                        okS1[:, j * 512 : (j + 1) * 512],
                        start=True,
                        stop=True,
                    )
                    nc.any.tensor_copy(
                        out=oksum_sb[:, j * 512 : (j + 1) * 512], in_=okm_ps[0:32, :]
                    )

                fpair_sb = small.tile([32, 256], F32, name="fpair_sb")
                fsum_pairs = fsum_rep[0:32].rearrange(
                    "p (hq two) -> p hq two", two=2
                )
                nc.vector.tensor_add(
                    fpair_sb[:], fsum_pairs[:, :, 0], fsum_pairs[:, :, 1]
                )

                okt = small.tile([32, 1], F32, name="okt")
                nc.vector.tensor_reduce(
                    okt[:], oksum_sb[:], axis=mybir.AxisListType.X,
                    op=mybir.AluOpType.add,
                )
                fst = small.tile([32, 1], F32, name="fst")
                nc.vector.tensor_reduce(
                    fst[:], fsum_rep[0:32], axis=mybir.AxisListType.X,
                    op=mybir.AluOpType.add,
                )
                prod = small.tile([32, 1], F32, name="prod")
                nc.vector.tensor_mul(prod[:], okt[:], fst[:])
                ones32 = small.tile([32, 1], F32, name="ones32")
                nc.vector.memset(ones32[:], 1.0)
                pi_ps = sps.tile([128, 512], F32, name="sps_t", tag="sps")
                nc.tensor.matmul(
                    pi_ps[0:1, 0:1], prod[:], ones32[:], start=True, stop=True
                )
                pi_sb = small.tile([1, 1], F32, name="pi_sb")
                nc.any.tensor_copy(out=pi_sb[:], in_=pi_ps[0:1, 0:1])

                cc_in = dram_pool.tile([1, 1], F32, name="cc_in")
                cc_out = dram_pool.tile([1, 1], F32, name="cc_out")
                nc.gpsimd.dma_start(cc_in[:], pi_sb[:])
                nc.gpsimd.collective_compute(
                    "AllReduce",
                    mybir.AluOpType.add,
                    replica_groups=[list(range(NCORES))],
                    ins=[cc_in.opt()],
                    outs=[cc_out.opt()],
                )

                akT = small.tile([128, 8, 256], F32, name="akT")
                for kc in range(8):
                    akT_ps = sps.tile([128, 512], F32, name="sps_t", tag="sps")
                    nc.tensor.matmul(
                        akT_ps[:, 0:256],
                        oksum_sb[:, kc * 128 : (kc + 1) * 128],
                        fpair_sb[:],
                        start=True,
                        stop=True,
                    )
                    nc.vector.tensor_copy(out=akT[:, kc, :], in_=akT_ps[:, 0:256])

                for s in range(2):
                    corr_group(1, s, split=False)

                # ---- side chain part 2: collective result -> avg_bc ----
                tot_sb = small.tile([1, 1], F32, name="tot_sb")
                nc.gpsimd.dma_start(tot_sb[:], cc_out[:])
                avg1 = small.tile([1, 1], F32, name="avg1")
                nc.vector.tensor_scalar_mul(avg1[:], tot_sb[:], 1.0 / AK_COUNT)
                ones_r = small.tile([1, 128], F32, name="ones_r")
                nc.vector.memset(ones_r[:], 1.0)
                avgb_ps = sps.tile([128, 512], F32, name="sps_t", tag="sps")
                nc.tensor.matmul(
                    avgb_ps[:, 0:1], ones_r[:], avg1[:], start=True, stop=True
                )
                avg_bc = small.tile([128, 1], F32, name="avg_bc")
                nc.any.tensor_copy(out=avg_bc[:], in_=avgb_ps[:, 0:1])

                for s in range(2):
                    corr_group(2, s, split=False)

                # ---- side chain part 3: mask/min/max/normalize + output
                mask = small.tile([128, 8, 256], F32, name="mask")
                nc.vector.tensor_scalar(
                    mask[:], akT[:], avg_bc[:], None, op0=mybir.AluOpType.is_gt
                )
                nc.vector.tensor_mul(akT[:], akT[:], mask[:])

                grp = akT.rearrange("p c (g h) -> p (c g) h", h=64)
                mins = small.tile([128, 32], F32, name="mins")
                nc.vector.tensor_reduce(
                    mins[:], grp, axis=mybir.AxisListType.X, op=mybir.AluOpType.min
                )
                for cg in range(32):
                    kc, g = divmod(cg, 4)
                    nc.vector.tensor_scalar(
                        akT[:, kc, g * 64 : (g + 1) * 64],
                        akT[:, kc, g * 64 : (g + 1) * 64],
                        mins[:, cg : cg + 1],
                        None,
                        op0=mybir.AluOpType.subtract,
                    )
                maxs = small.tile([128, 32], F32, name="maxs")
                nc.vector.tensor_reduce(
                    maxs[:], grp, axis=mybir.AxisListType.X, op=mybir.AluOpType.max
                )
                rec = small.tile([128, 32], F32, name="rec")
                nc.vector.reciprocal(rec[:], maxs[:])
                for cg in range(32):
                    kc, g = divmod(cg, 4)
                    nc.vector.tensor_scalar(
                        akT[:, kc, g * 64 : (g + 1) * 64],
                        akT[:, kc, g * 64 : (g + 1) * 64],
                        rec[:, cg : cg + 1],
                        None,
                        op0=mybir.AluOpType.mult,
                    )

                akh_r0 = small.tile([128, 1024], F32, name="akh_r0")
                akh_r1 = small.tile([128, 1024], F32, name="akh_r1")
                for kc in range(8):
                    for half, dst in ((0, akh_r0), (1, akh_r1)):
                        t_ps = sps.tile([128, 512], F32, name="sps_t", tag="sps")
                        nc.tensor.transpose(
                            t_ps[:, 0:128],
                            akT[:, kc, half * 128 : (half + 1) * 128],
                            ident_sb[:],
                        )
                        nc.any.tensor_copy(
                            out=dst[:, kc * 128 : (kc + 1) * 128], in_=t_ps[:, 0:128]
                        )
                nc.gpsimd.dma_start(akhat_out[0:128, :], akh_r0[:])
                nc.gpsimd.dma_start(akhat_out[128:256, :], akh_r1[:])

                for s in range(2):
                    corr_group(3, s, split=False)

    nc.compile()
    return nc


_CACHE = {}


def _get_nc():
    if "nc" not in _CACHE:
        _CACHE["nc"] = _build_kernel()
    return _CACHE["nc"]


def _make_consts():
    sel4r = np.zeros((4, 128, 128), dtype=np.float32)
    b = np.arange(32)
    for hh in range(4):
        for x in range(4):
            sel4r[hh, b * 4 + hh, x * 32 + b] = 1.0
    selb = np.tile(np.eye(32, dtype=np.float32), (4, 1))
    ident = np.eye(128, dtype=np.float32)
    return sel4r, selb, ident


def kernel(ok, fk):
    global LAST_RESULTS
    import os

    ok = np.ascontiguousarray(np.asarray(ok), dtype=np.float32)
    fk = np.ascontiguousarray(np.asarray(fk), dtype=np.float32)
    assert ok.shape == (B, 32, 32, 32) and fk.shape == (B, H, W, D)

    okB = np.ascontiguousarray(
        ok.reshape(32, 4, 8192).transpose(1, 0, 2).reshape(128, 8192)
    )
    sel4r, selb, ident = _make_consts()

    in_maps = []
    for i in range(NCORES):
        fkT = np.ascontiguousarray(fk[:, 8 * i : 8 * (i + 1)]).reshape(128, 8192)
        in_maps.append(
            {"okB": okB, "fkT": fkT, "sel4r": sel4r, "selb": selb, "ident": ident}
        )

    nc = _get_nc()
    res = run_bass_kernel_spmd(
        nc,
        in_maps,
        list(range(NCORES)),
        trace=bool(int(os.environ.get("KERNEL_TRACE", "0"))),
    )
    LAST_RESULTS = res

    corr = np.empty((64, 64 * 32768), dtype=np.float32)
    akh = np.empty((32, 64, 1024), dtype=np.float32)
    for i in range(NCORES):
        r = res.results[i]
        corr[8 * i : 8 * (i + 1)] = r["corr_out"].reshape(8, 64 * 32768)
        akh[4 * i : 4 * (i + 1)] = r["akhat_out"].reshape(4, 64, 1024)
    corr = corr.reshape(B, H, W, 1024)
    return corr, akh
